# revision 17
# baseline (speedup 1.0000x reference)
"""Trainium2 Bass kernel for the differentiable compressor.

Algorithm
---------
The time recurrence  s_t = a_t s_{t-1} + (1-a_t) v_t,
a_t = A_AT if v_t > s_{t-1} else A_REL  is a max-linear system; policy
iteration (guess modes, solve the linear recurrence exactly with the
hardware tensor_tensor_scan, repeat) converges geometrically.  4 total
iterations reach ~7.5e-4 output rel err (gate 2e-2).

Everything runs in natural-log units (v = ln(|x|+1e-8)).  Trajectory kept
relative to the input, r_t = s_t - v_t, so the scan is
    r_t = a_t * (r_{t-1} + delta_t),   delta_t = v_{t-1} - v_t,
with delta precomputed once.  Key identity: sign(r_t) = sign(r_{t-1} +
delta_t), so the next iteration's modes come straight from the previous
trajectory's sign: a_t = MID - HDA * sign(r_t_prev), computed entirely on
the Scalar engine (Sign + Copy-affine) and hidden under the scans.

Gain stage: the knee-smoothed down/up gains collapse (to ~1e-4 rel) to
    g = c1*w + c2*|w|,  c1 = CDN-CUP, c2 = CDN+CUP,  w = r + v - th,
clamped at UP_RANGE on the up side (applied after exp, where it is a min
against e^{dep*UPR}).  Per quarter: TT (u = r+v), ACT Abs (bias=-th),
STT (gg = (c2/c1)|w| + u), ACT Exp (scale=c1*dep, bias=-c1*dep*th), and
STT ((eg min clamp) * x).  All activations used (Ln, Exp, Abs, Sign,
Copy) live in one table set -> one ACT_TABLE_LOAD total.

Layout per core: 2 batch rows x 441000 samples -> [126 partitions x 7000],
63 time-chunks per row.  Chunk-boundary carries live in an extra leading
column of the trajectory tile; between iterations the carry shift
(partition p-1 -> p) is done by the idle Tensor engine with a host-
supplied shift matrix into PSUM, off the DVE critical path.  The final
iteration also uses lagged carries (exact-carry fixup unneeded).

Sharding: pure data parallel, batch 16 -> 2 rows on each of 8 cores.
"""
import sys
import types
import numpy as np

# ---------------- constants (natural-log units) ----------------
SR = 44100.0
A_AT = float(np.exp(-1.0 / (10.0 * SR / 1000.0)))     # attack coeff
A_REL = float(np.exp(-1.0 / (100.0 * SR / 1000.0)))   # release coeff
DA = A_AT - A_REL
MID = (A_AT + A_REL) / 2.0
HDA = (A_AT - A_REL) / 2.0
CNAT = float(np.log(10.0) / 20.0)                     # dB -> nat
CDN = -(1.0 - 1.0 / 66.7) * 0.5                       # down-ratio gain slope
CUP = (1.0 - 0.1) * 0.5                               # up-ratio gain slope
C1 = CDN - CUP
C2 = CDN + CUP
C21 = C2 / C1
ALF = (C1 + C2) / (C1 - C2)   # Prelu negative-side slope = -CDN/CUP
UPR = 36.0 * CNAT                                     # up-range clamp
TMIN, TMAX = -40.0, 0.0

B, N = 16, 441000
NCORES = 8
ROWS = 2           # batch rows per core
NCH = 63           # chunks per row
P = ROWS * NCH     # 126 partitions
L = N // NCH       # 7000 chunk length
H = L // 2         # half-width for engine overlap
Q = L // 4         # quarter-width for the gain tail

NITER = 4          # policy iterations (incl. the first delta-sign one)
NS = 10            # x streaming slices (700 cols each, even for 2x DVE mode)
CW = L // NS


def _install_ntff_hook():
    """Inject the missing antenv.axon_hooks so trace=True profiling works."""
    try:
        import antenv
        if "antenv.axon_hooks" not in sys.modules:
            m = types.ModuleType("antenv.axon_hooks")
            m._hook = None
            def _set(h, _m=m): _m._hook = h
            def _get(_m=m): return _m._hook
            m.set_axon_ntff_profile_hook = _set
            m.get_axon_ntff_profile_hook = _get
            sys.modules["antenv.axon_hooks"] = m
            antenv.axon_hooks = m
            from trn_agent_boot.trn_boot import _ntff_profile_via_ctypes
            _set(_ntff_profile_via_ctypes("/opt/axon/libaxon_pjrt.so"))
    except Exception:
        pass


def build_nc():
    import concourse.bacc as bacc
    import concourse.mybir as mybir
    from concourse.tile import TileContext
    from concourse.alu_op_type import AluOpType as Op
    AF = mybir.ActivationFunctionType

    nc = bacc.Bacc("TRN2", target_bir_lowering=False, debug=False)
    x_d = nc.dram_tensor("x", [P, L], mybir.dt.float32, kind="ExternalInput")
    aux_d = nc.dram_tensor("aux", [P, 8], mybir.dt.float32, kind="ExternalInput")
    shm_d = nc.dram_tensor("shm", [P, P], mybir.dt.float32, kind="ExternalInput")
    y_d = nc.dram_tensor("y", [P, L], mybir.dt.float32, kind="ExternalOutput")

    f32 = mybir.dt.float32
    with TileContext(nc) as tc:
        with tc.tile_pool(name="pool", bufs=1) as pool, \
             tc.tile_pool(name="psum", bufs=1, space="PSUM") as psum:
            tx = pool.tile([P, L], f32)        # x (resident; used at the end)
            tv = pool.tile([P, L], f32)        # v; post: u = r + v
            tD = pool.tile([P, L], f32)        # delta; post: |w|, exp scratch
            tse = pool.tile([P, L + 1], f32)   # r trajectory, col0 = carry
            ta = pool.tile([P, L], f32)        # coefficients a; post: gg, y
            taux = pool.tile([P, 8], f32)      # host-computed columns
            tshm = pool.tile([P, P], f32)      # carry shift matrix (PE)
            pinit = psum.tile([P, 1], f32)     # shifted carries via PE

            LNB = taux[:, 0:1]    # 1e-8 (Ln bias)
            PRB = taux[:, 1:2]    # 2*CUP*th (Prelu bias)
            EXS = taux[:, 2:3]    # dep (Exp scale)
            CLK = taux[:, 4:5]    # exp(dep*UPR) clamp
            TCL = taux[:, 5:6]    # v at end of previous chunk (host ln)
            SC2 = taux[:, 7:8]    # scratch

            nc.sync.dma_start(taux[:], aux_d[:])
            for j in range(NS):
                sl = slice(j * CW, (j + 1) * CW)
                nc.sync.dma_start(tx[:, sl], x_d[:, sl])
            nc.sync.dma_start(tshm[:], shm_d[:])

            # scan-carry column starts at 0 (r_{-1} = 0)
            nc.vector.memset(tse[:, 0:1], 0.0)

            # streamed: v' = ln(x^2 + 1e-16) = 2*ln(|x|+1e-8) on ACT only
            # (Square then Ln; the factor 2 is absorbed exactly into the
            # downstream constants), delta on DVE, a0 = A_REL + DA*[delta<0]
            # on DVE per half as soon as its slices are in.
            for j in range(NS):
                sl = slice(j * CW, (j + 1) * CW)
                nc.scalar.activation(tv[:, sl], tx[:, sl], AF.Square, bias=0.0, scale=1.0)
                nc.scalar.activation(tv[:, sl], tv[:, sl], AF.Ln, bias=LNB, scale=1.0)
                lo = j * CW
                s_in = slice(lo if j else 1, (j + 1) * CW)
                s_sh = slice((lo - 1) if j else 0, (j + 1) * CW - 1)
                nc.vector.tensor_tensor(tD[:, s_in], tv[:, s_sh], tv[:, s_in],
                                        Op.subtract)
                if j == 0:
                    # cross-chunk delta col 0 from the host-computed column
                    nc.vector.tensor_tensor(tD[:, 0:1], TCL, tv[:, 0:1],
                                            Op.subtract)
                if j == NS // 2 - 1 or j == NS - 1:
                    hs = slice(0, H) if j == NS // 2 - 1 else slice(H, L)
                    nc.vector.tensor_scalar(ta[:, hs], tD[:, hs], 0.0, None,
                                            op0=Op.is_lt)
                    nc.vector.tensor_scalar(ta[:, hs], ta[:, hs], DA, A_REL,
                                            op0=Op.mult, op1=Op.add)

            # dummy Exp chained after the last Ln: pulls the switch to the
            # exp_and_others table set (which also has Sign/Copy/Abs) into
            # the ACT idle window instead of the gain tail
            nc.scalar.activation(SC2, tv[:, L - 1:L], AF.Exp, bias=0.0, scale=0.0)

            for it in range(NITER - 1):
                init = tse[:, 0:1] if it == 0 else pinit[:]
                nc.vector.tensor_tensor_scan(
                    tse[:, 1:H + 1], tD[:, 0:H], ta[:, 0:H], init,
                    op0=Op.add, op1=Op.mult)
                # next modes for h1 overlap the h2 scan on the DVE
                nc.scalar.activation(ta[:, 0:H], tse[:, 1:H + 1], AF.Sign,
                                     bias=0.0, scale=1.0)
                nc.scalar.activation(ta[:, 0:H], ta[:, 0:H], AF.Copy,
                                     bias=MID, scale=-HDA)
                nc.vector.tensor_tensor_scan(
                    tse[:, H + 1:L + 1], tD[:, H:L], ta[:, H:L], tse[:, H:H + 1],
                    op0=Op.add, op1=Op.mult)
                nc.scalar.activation(ta[:, H:L], tse[:, H + 1:L + 1], AF.Sign,
                                     bias=0.0, scale=1.0)
                nc.scalar.activation(ta[:, H:L], ta[:, H:L], AF.Copy,
                                     bias=MID, scale=-HDA)
                # lagged chunk carries: shift partition p-1 -> p on the
                # idle Tensor engine (rows 0/63 get 0), result in PSUM
                nc.tensor.matmul(pinit[:], tshm[:], tse[:, L:L + 1])

            # ------------- final iteration + gain tail, interleaved -------
            #   u = r + v;  w = u - th
            #   g = c1*w + c2*|w|  ==  Prelu(-2*CUP*w; alpha=-CDN/CUP)
            #   y = min(exp(dep*g), e^{dep*UPR}) * x
            # The last iteration's scan runs in quarters; each quarter's
            # gain chain (TT u -> ACT Prelu -> ACT Exp -> STT y -> DMA)
            # starts while later quarters are still scanning, so the y
            # output DMA streams during the final scans.
            # piece boundaries: big pieces early, small pieces last so the
            # final serial chain (scan->u->Prelu->Exp->y) is short; y STTs
            # for piece i-1 are emitted after piece i's scan+u so the next
            # scan always has DVE priority over y production.
            cuts = [0, 1750, 3500, 5250, 6126, 7000]
            pend = []   # y pieces whose Exp is in flight
            for i in range(len(cuts) - 1):
                lo, hi = cuts[i], cuts[i + 1]
                sl = slice(lo, hi)
                init = pinit[:] if i == 0 else tse[:, lo:lo + 1]
                nc.vector.tensor_tensor_scan(
                    tse[:, lo + 1:hi + 1], tD[:, sl], ta[:, sl], init,
                    op0=Op.add, op1=Op.mult)
                nc.vector.tensor_tensor(tv[:, sl], tse[:, lo + 1:hi + 1],
                                        tv[:, sl], Op.add)
                nc.scalar.activation(tD[:, sl], tv[:, sl], AF.Prelu,
                                     bias=PRB, scale=-CUP, alpha=ALF)
                nc.scalar.activation(tD[:, sl], tD[:, sl], AF.Exp, bias=0.0, scale=EXS)
                if hi - lo > 1000:
                    pend.append(slice(lo, (lo + hi) // 2))
                    pend.append(slice((lo + hi) // 2, hi))
                else:
                    pend.append(sl)
                if i >= 1:
                    se = pend.pop(0)
                    nc.vector.scalar_tensor_tensor(
                        ta[:, se], tD[:, se], CLK, tx[:, se], op0=Op.min, op1=Op.mult)
                    nc.sync.dma_start(y_d[:, se], ta[:, se])
            for se in pend:
                nc.vector.scalar_tensor_tensor(
                    ta[:, se], tD[:, se], CLK, tx[:, se], op0=Op.min, op1=Op.mult)
                nc.sync.dma_start(y_d[:, se], ta[:, se])

    nc.compile()
    return nc


_NC = None


def _get_nc():
    global _NC
    if _NC is None:
        _NC = build_nc()
    return _NC


def _shift_matrix():
    """W[k, p] = 1 iff p = k+1 within a row's chunk run (chunk 0 gets 0)."""
    w = np.zeros((P, P), np.float32)
    for p in range(P):
        if p % NCH != 0:
            w[p - 1, p] = 1.0
    return w


_SHM = _shift_matrix()


def make_in_maps(x, threshold, depth):
    th_nat = ((TMIN + threshold.astype(np.float32) * (TMAX - TMIN)) *
              np.float32(CNAT)).astype(np.float32)           # [16,1]
    dep = depth.astype(np.float32)
    aux_full = np.zeros((B, 8), np.float32)
    aux_full[:, 0] = 1e-16
    aux_full[:, 1] = np.float32(2.0 * CUP) * th_nat[:, 0]
    aux_full[:, 2] = dep[:, 0]
    aux_full[:, 4] = np.exp(dep[:, 0] * np.float32(UPR))
    in_maps = []
    for i in range(NCORES):
        xs = np.ascontiguousarray(x[ROWS * i:ROWS * (i + 1)]).reshape(P, L)
        auxs = np.repeat(aux_full[ROWS * i:ROWS * (i + 1)], NCH, axis=0)
        # host-computed v at end of previous chunk (chunk 0: own col 0,
        # so delta col0 = 0 there)
        vend = np.log(xs[:, L - 1] ** 2 + np.float32(1e-16)).astype(np.float32)
        tcl = np.empty(P, np.float32)
        tcl[1:] = vend[:-1]
        for r0 in (0, NCH):
            tcl[r0] = np.log(xs[r0, 0] ** 2 + np.float32(1e-16))
        auxs = np.ascontiguousarray(auxs, np.float32)
        auxs[:, 5] = tcl
        in_maps.append({"x": xs.astype(np.float32), "aux": auxs, "shm": _SHM})
    return in_maps


def kernel(x, threshold, depth):
    _install_ntff_hook()
    from concourse.bass_utils import run_bass_kernel_spmd
    nc = _get_nc()
    x = np.asarray(x, np.float32)
    in_maps = make_in_maps(x, np.asarray(threshold), np.asarray(depth))
    res = run_bass_kernel_spmd(nc, in_maps, core_ids=list(range(NCORES)))
    y = np.empty((B, N), np.float32)
    for i in range(NCORES):
        y[ROWS * i:ROWS * (i + 1)] = np.asarray(res.results[i]["y"]).reshape(ROWS, N)
    return y


# revision 19
# speedup vs baseline: 1.0461x; 1.0461x over previous
"""Trainium2 Bass kernel for the differentiable compressor.

Algorithm
---------
The time recurrence  s_t = a_t s_{t-1} + (1-a_t) v_t,
a_t = A_AT if v_t > s_{t-1} else A_REL  is a max-linear system; policy
iteration (guess modes, solve the linear recurrence exactly with the
hardware tensor_tensor_scan, repeat) converges geometrically.  4 total
iterations reach ~7.5e-4 output rel err (gate 2e-2).

Everything runs in natural-log units (v = ln(|x|+1e-8)).  Trajectory kept
relative to the input, r_t = s_t - v_t, so the scan is
    r_t = a_t * (r_{t-1} + delta_t),   delta_t = v_{t-1} - v_t,
with delta precomputed once.  Key identity: sign(r_t) = sign(r_{t-1} +
delta_t), so the next iteration's modes come straight from the previous
trajectory's sign: a_t = MID - HDA * sign(r_t_prev), computed entirely on
the Scalar engine (Sign + Copy-affine) and hidden under the scans.

Gain stage: the knee-smoothed down/up gains collapse (to ~1e-4 rel) to
    g = c1*w + c2*|w|,  c1 = CDN-CUP, c2 = CDN+CUP,  w = r + v - th,
clamped at UP_RANGE on the up side (applied after exp, where it is a min
against e^{dep*UPR}).  Per quarter: TT (u = r+v), ACT Abs (bias=-th),
STT (gg = (c2/c1)|w| + u), ACT Exp (scale=c1*dep, bias=-c1*dep*th), and
STT ((eg min clamp) * x).  All activations used (Ln, Exp, Abs, Sign,
Copy) live in one table set -> one ACT_TABLE_LOAD total.

Layout per core: 2 batch rows x 441000 samples -> [126 partitions x 7000],
63 time-chunks per row.  Chunk-boundary carries live in an extra leading
column of the trajectory tile; between iterations the carry shift
(partition p-1 -> p) is done by the idle Tensor engine with a host-
supplied shift matrix into PSUM, off the DVE critical path.  The final
iteration also uses lagged carries (exact-carry fixup unneeded).

Sharding: pure data parallel, batch 16 -> 2 rows on each of 8 cores.
"""
import sys
import types
import numpy as np

# ---------------- constants (natural-log units) ----------------
SR = 44100.0
A_AT = float(np.exp(-1.0 / (10.0 * SR / 1000.0)))     # attack coeff
A_REL = float(np.exp(-1.0 / (100.0 * SR / 1000.0)))   # release coeff
DA = A_AT - A_REL
MID = (A_AT + A_REL) / 2.0
HDA = (A_AT - A_REL) / 2.0
CNAT = float(np.log(10.0) / 20.0)                     # dB -> nat
CDN = -(1.0 - 1.0 / 66.7) * 0.5                       # down-ratio gain slope
CUP = (1.0 - 0.1) * 0.5                               # up-ratio gain slope
C1 = CDN - CUP
C2 = CDN + CUP
C21 = C2 / C1
ALF = (C1 + C2) / (C1 - C2)   # Prelu negative-side slope = -CDN/CUP
UPR = 36.0 * CNAT                                     # up-range clamp
TMIN, TMAX = -40.0, 0.0

B, N = 16, 441000
NCORES = 8
ROWS = 2           # batch rows per core
NCH = 63           # chunks per row
P = ROWS * NCH     # 126 partitions
L = N // NCH       # 7000 chunk length
H = L // 2         # half-width for engine overlap
Q = L // 4         # quarter-width for the gain tail

NITER = 4          # policy iterations (incl. the first delta-sign one)
NS = 10            # x streaming slices (700 cols each, even for 2x DVE mode)
CW = L // NS


def _install_ntff_hook():
    """Inject the missing antenv.axon_hooks so trace=True profiling works."""
    try:
        import antenv
        if "antenv.axon_hooks" not in sys.modules:
            m = types.ModuleType("antenv.axon_hooks")
            m._hook = None
            def _set(h, _m=m): _m._hook = h
            def _get(_m=m): return _m._hook
            m.set_axon_ntff_profile_hook = _set
            m.get_axon_ntff_profile_hook = _get
            sys.modules["antenv.axon_hooks"] = m
            antenv.axon_hooks = m
            from trn_agent_boot.trn_boot import _ntff_profile_via_ctypes
            _set(_ntff_profile_via_ctypes("/opt/axon/libaxon_pjrt.so"))
    except Exception:
        pass


def build_nc():
    import concourse.bacc as bacc
    import concourse.mybir as mybir
    from concourse.tile import TileContext
    from concourse.alu_op_type import AluOpType as Op
    AF = mybir.ActivationFunctionType

    nc = bacc.Bacc("TRN2", target_bir_lowering=False, debug=False)
    x_d = nc.dram_tensor("x", [P, L], mybir.dt.float32, kind="ExternalInput")
    aux_d = nc.dram_tensor("aux", [P, 8], mybir.dt.float32, kind="ExternalInput")
    shm_d = nc.dram_tensor("shm", [P, P], mybir.dt.float32, kind="ExternalInput")
    y_d = nc.dram_tensor("y", [P, L], mybir.dt.float32, kind="ExternalOutput")

    f32 = mybir.dt.float32
    with TileContext(nc) as tc:
        with tc.tile_pool(name="pool", bufs=1) as pool, \
             tc.tile_pool(name="psum", bufs=1, space="PSUM") as psum:
            tx = pool.tile([P, L], f32)        # x (resident; used at the end)
            tv = pool.tile([P, L], f32)        # v; post: u = r + v
            tD = pool.tile([P, L], f32)        # delta; post: |w|, exp scratch
            tse = pool.tile([P, L + 1], f32)   # r trajectory, col0 = carry
            ta = pool.tile([P, L], f32)        # coefficients a; post: gg, y
            taux = pool.tile([P, 8], f32)      # host-computed columns
            tshm = pool.tile([P, P], f32)      # carry shift matrix (PE)
            pinit = psum.tile([P, 1], f32)     # shifted carries via PE

            LNB = taux[:, 0:1]    # 1e-8 (Ln bias)
            PRB = taux[:, 1:2]    # 2*CUP*th (Prelu bias)
            EXS = taux[:, 2:3]    # dep (Exp scale)
            CLK = taux[:, 4:5]    # exp(dep*UPR) clamp
            TCL = taux[:, 5:6]    # v at end of previous chunk (host ln)
            SC2 = taux[:, 7:8]    # scratch

            nc.sync.dma_start(taux[:], aux_d[:])
            for j in range(NS):
                sl = slice(j * CW, (j + 1) * CW)
                nc.sync.dma_start(tx[:, sl], x_d[:, sl])
            nc.sync.dma_start(tshm[:], shm_d[:])

            # preload the natural_log table set (contains Square/Sign/Copy
            # too) before the first streamed Square lands
            nc.scalar.activation(SC2, taux[:, 6:7], AF.Ln, bias=LNB, scale=1.0)

            # scan-carry column starts at 0 (r_{-1} = 0)
            nc.vector.memset(tse[:, 0:1], 0.0)

            # streamed: v' = ln(x^2 + 1e-16) = 2*ln(|x|+1e-8) on ACT only
            # (Square then Ln; the factor 2 is absorbed exactly into the
            # downstream constants), delta on DVE, a0 = A_REL + DA*[delta<0]
            # on DVE per half as soon as its slices are in.
            for j in range(NS):
                sl = slice(j * CW, (j + 1) * CW)
                nc.scalar.activation(tv[:, sl], tx[:, sl], AF.Square, bias=0.0, scale=1.0)
                nc.scalar.activation(tv[:, sl], tv[:, sl], AF.Ln, bias=LNB, scale=1.0)
                lo = j * CW
                s_in = slice(lo if j else 1, (j + 1) * CW)
                s_sh = slice((lo - 1) if j else 0, (j + 1) * CW - 1)
                nc.vector.tensor_tensor(tD[:, s_in], tv[:, s_sh], tv[:, s_in],
                                        Op.subtract)
                if j == 0:
                    # cross-chunk delta col 0 from the host-computed column
                    nc.vector.tensor_tensor(tD[:, 0:1], TCL, tv[:, 0:1],
                                            Op.subtract)
                if j == NS // 2 - 1:
                    # a0 first half on DVE (idle while slices stream in)
                    nc.vector.tensor_scalar(ta[:, 0:H], tD[:, 0:H], 0.0, None,
                                            op0=Op.is_lt)
                    nc.vector.tensor_scalar(ta[:, 0:H], ta[:, 0:H], DA, A_REL,
                                            op0=Op.mult, op1=Op.add)
                if j == NS - 1:
                    # a0 second half on ACT: delta h2 is complete before
                    # scan h1 finishes, so this hides in the ACT idle window
                    nc.scalar.activation(ta[:, H:L], tD[:, H:L], AF.Sign,
                                         bias=0.0, scale=1.0)
                    nc.scalar.activation(ta[:, H:L], ta[:, H:L], AF.Copy,
                                         bias=MID, scale=-HDA)

            # dummy Exp chained after the last Ln: pulls the switch to the
            # exp_and_others table set (which also has Sign/Copy/Abs) into
            # the ACT idle window instead of the gain tail
            nc.scalar.activation(SC2, tv[:, L - 1:L], AF.Exp, bias=0.0, scale=0.0)

            for it in range(NITER - 1):
                init = tse[:, 0:1] if it == 0 else pinit[:]
                nc.vector.tensor_tensor_scan(
                    tse[:, 1:H + 1], tD[:, 0:H], ta[:, 0:H], init,
                    op0=Op.add, op1=Op.mult)
                # next modes for h1 overlap the h2 scan on the DVE
                nc.scalar.activation(ta[:, 0:H], tse[:, 1:H + 1], AF.Sign,
                                     bias=0.0, scale=1.0)
                nc.scalar.activation(ta[:, 0:H], ta[:, 0:H], AF.Copy,
                                     bias=MID, scale=-HDA)
                nc.vector.tensor_tensor_scan(
                    tse[:, H + 1:L + 1], tD[:, H:L], ta[:, H:L], tse[:, H:H + 1],
                    op0=Op.add, op1=Op.mult)
                nc.scalar.activation(ta[:, H:L], tse[:, H + 1:L + 1], AF.Sign,
                                     bias=0.0, scale=1.0)
                nc.scalar.activation(ta[:, H:L], ta[:, H:L], AF.Copy,
                                     bias=MID, scale=-HDA)
                # lagged chunk carries: shift partition p-1 -> p on the
                # idle Tensor engine (rows 0/63 get 0), result in PSUM
                nc.tensor.matmul(pinit[:], tshm[:], tse[:, L:L + 1])

            # ------------- final iteration + gain tail, interleaved -------
            #   u = r + v;  w = u - th
            #   g = c1*w + c2*|w|  ==  Prelu(-2*CUP*w; alpha=-CDN/CUP)
            #   y = min(exp(dep*g), e^{dep*UPR}) * x
            # The last iteration's scan runs in quarters; each quarter's
            # gain chain (TT u -> ACT Prelu -> ACT Exp -> STT y -> DMA)
            # starts while later quarters are still scanning, so the y
            # output DMA streams during the final scans.
            # piece boundaries: big pieces early, small pieces last so the
            # final serial chain (scan->u->Prelu->Exp->y) is short; y STTs
            # for piece i-1 are emitted after piece i's scan+u so the next
            # scan always has DVE priority over y production.
            cuts = [0, 1750, 3500, 5250, 6126, 7000]
            pend = []   # y pieces whose Exp is in flight
            for i in range(len(cuts) - 1):
                lo, hi = cuts[i], cuts[i + 1]
                sl = slice(lo, hi)
                init = pinit[:] if i == 0 else tse[:, lo:lo + 1]
                nc.vector.tensor_tensor_scan(
                    tse[:, lo + 1:hi + 1], tD[:, sl], ta[:, sl], init,
                    op0=Op.add, op1=Op.mult)
                nc.vector.tensor_tensor(tv[:, sl], tse[:, lo + 1:hi + 1],
                                        tv[:, sl], Op.add)
                nc.scalar.activation(tD[:, sl], tv[:, sl], AF.Prelu,
                                     bias=PRB, scale=-CUP, alpha=ALF)
                nc.scalar.activation(tD[:, sl], tD[:, sl], AF.Exp, bias=0.0, scale=EXS)
                if hi - lo > 1000:
                    pend.append(slice(lo, (lo + hi) // 2))
                    pend.append(slice((lo + hi) // 2, hi))
                else:
                    pend.append(sl)
                if i >= 1:
                    se = pend.pop(0)
                    nc.vector.scalar_tensor_tensor(
                        ta[:, se], tD[:, se], CLK, tx[:, se], op0=Op.min, op1=Op.mult)
                    nc.sync.dma_start(y_d[:, se], ta[:, se])
            for se in pend:
                nc.vector.scalar_tensor_tensor(
                    ta[:, se], tD[:, se], CLK, tx[:, se], op0=Op.min, op1=Op.mult)
                nc.sync.dma_start(y_d[:, se], ta[:, se])

    nc.compile()
    return nc


_NC = None


def _get_nc():
    global _NC
    if _NC is None:
        _NC = build_nc()
    return _NC


def _shift_matrix():
    """W[k, p] = 1 iff p = k+1 within a row's chunk run (chunk 0 gets 0)."""
    w = np.zeros((P, P), np.float32)
    for p in range(P):
        if p % NCH != 0:
            w[p - 1, p] = 1.0
    return w


_SHM = _shift_matrix()


def make_in_maps(x, threshold, depth):
    th_nat = ((TMIN + threshold.astype(np.float32) * (TMAX - TMIN)) *
              np.float32(CNAT)).astype(np.float32)           # [16,1]
    dep = depth.astype(np.float32)
    aux_full = np.zeros((B, 8), np.float32)
    aux_full[:, 0] = 1e-16
    aux_full[:, 1] = np.float32(2.0 * CUP) * th_nat[:, 0]
    aux_full[:, 2] = dep[:, 0]
    aux_full[:, 4] = np.exp(dep[:, 0] * np.float32(UPR))
    in_maps = []
    for i in range(NCORES):
        xs = np.ascontiguousarray(x[ROWS * i:ROWS * (i + 1)]).reshape(P, L)
        auxs = np.repeat(aux_full[ROWS * i:ROWS * (i + 1)], NCH, axis=0)
        # host-computed v at end of previous chunk (chunk 0: own col 0,
        # so delta col0 = 0 there)
        vend = np.log(xs[:, L - 1] ** 2 + np.float32(1e-16)).astype(np.float32)
        tcl = np.empty(P, np.float32)
        tcl[1:] = vend[:-1]
        for r0 in (0, NCH):
            tcl[r0] = np.log(xs[r0, 0] ** 2 + np.float32(1e-16))
        auxs = np.ascontiguousarray(auxs, np.float32)
        auxs[:, 5] = tcl
        in_maps.append({"x": xs.astype(np.float32), "aux": auxs, "shm": _SHM})
    return in_maps


def kernel(x, threshold, depth):
    _install_ntff_hook()
    from concourse.bass_utils import run_bass_kernel_spmd
    nc = _get_nc()
    x = np.asarray(x, np.float32)
    in_maps = make_in_maps(x, np.asarray(threshold), np.asarray(depth))
    res = run_bass_kernel_spmd(nc, in_maps, core_ids=list(range(NCORES)))
    y = np.empty((B, N), np.float32)
    for i in range(NCORES):
        y[ROWS * i:ROWS * (i + 1)] = np.asarray(res.results[i]["y"]).reshape(ROWS, N)
    return y


# revision 21
# speedup vs baseline: 1.0503x; 1.0041x over previous
"""Trainium2 Bass kernel for the differentiable compressor.

Algorithm
---------
The time recurrence  s_t = a_t s_{t-1} + (1-a_t) v_t,
a_t = A_AT if v_t > s_{t-1} else A_REL  is a max-linear system; policy
iteration (guess modes, solve the linear recurrence exactly with the
hardware tensor_tensor_scan, repeat) converges geometrically.  4 total
iterations reach ~7.5e-4 output rel err (gate 2e-2).

Everything runs in natural-log units (v = ln(|x|+1e-8)).  Trajectory kept
relative to the input, r_t = s_t - v_t, so the scan is
    r_t = a_t * (r_{t-1} + delta_t),   delta_t = v_{t-1} - v_t,
with delta precomputed once.  Key identity: sign(r_t) = sign(r_{t-1} +
delta_t), so the next iteration's modes come straight from the previous
trajectory's sign: a_t = MID - HDA * sign(r_t_prev), computed entirely on
the Scalar engine (Sign + Copy-affine) and hidden under the scans.

Gain stage: the knee-smoothed down/up gains collapse (to ~1e-4 rel) to
    g = c1*w + c2*|w|,  c1 = CDN-CUP, c2 = CDN+CUP,  w = r + v - th,
clamped at UP_RANGE on the up side (applied after exp, where it is a min
against e^{dep*UPR}).  Per quarter: TT (u = r+v), ACT Abs (bias=-th),
STT (gg = (c2/c1)|w| + u), ACT Exp (scale=c1*dep, bias=-c1*dep*th), and
STT ((eg min clamp) * x).  All activations used (Ln, Exp, Abs, Sign,
Copy) live in one table set -> one ACT_TABLE_LOAD total.

Layout per core: 2 batch rows x 441000 samples -> [126 partitions x 7000],
63 time-chunks per row.  Chunk-boundary carries live in an extra leading
column of the trajectory tile; between iterations the carry shift
(partition p-1 -> p) is done by the idle Tensor engine with a host-
supplied shift matrix into PSUM, off the DVE critical path.  The final
iteration also uses lagged carries (exact-carry fixup unneeded).

Sharding: pure data parallel, batch 16 -> 2 rows on each of 8 cores.
"""
import sys
import types
import numpy as np

# ---------------- constants (natural-log units) ----------------
SR = 44100.0
A_AT = float(np.exp(-1.0 / (10.0 * SR / 1000.0)))     # attack coeff
A_REL = float(np.exp(-1.0 / (100.0 * SR / 1000.0)))   # release coeff
DA = A_AT - A_REL
MID = (A_AT + A_REL) / 2.0
HDA = (A_AT - A_REL) / 2.0
CNAT = float(np.log(10.0) / 20.0)                     # dB -> nat
CDN = -(1.0 - 1.0 / 66.7) * 0.5                       # down-ratio gain slope
CUP = (1.0 - 0.1) * 0.5                               # up-ratio gain slope
C1 = CDN - CUP
C2 = CDN + CUP
C21 = C2 / C1
ALF = (C1 + C2) / (C1 - C2)   # Prelu negative-side slope = -CDN/CUP
UPR = 36.0 * CNAT                                     # up-range clamp
TMIN, TMAX = -40.0, 0.0

B, N = 16, 441000
NCORES = 8
ROWS = 2           # batch rows per core
NCH = 63           # chunks per row
P = ROWS * NCH     # 126 partitions
L = N // NCH       # 7000 chunk length
H = L // 2         # half-width for engine overlap
Q = L // 4         # quarter-width for the gain tail

NITER = 4          # policy iterations (incl. the first delta-sign one)
NS = 10            # x streaming slices (700 cols each, even for 2x DVE mode)
CW = L // NS


def _install_ntff_hook():
    """Inject the missing antenv.axon_hooks so trace=True profiling works."""
    try:
        import antenv
        if "antenv.axon_hooks" not in sys.modules:
            m = types.ModuleType("antenv.axon_hooks")
            m._hook = None
            def _set(h, _m=m): _m._hook = h
            def _get(_m=m): return _m._hook
            m.set_axon_ntff_profile_hook = _set
            m.get_axon_ntff_profile_hook = _get
            sys.modules["antenv.axon_hooks"] = m
            antenv.axon_hooks = m
            from trn_agent_boot.trn_boot import _ntff_profile_via_ctypes
            _set(_ntff_profile_via_ctypes("/opt/axon/libaxon_pjrt.so"))
    except Exception:
        pass


def build_nc():
    import concourse.bacc as bacc
    import concourse.mybir as mybir
    from concourse.tile import TileContext
    from concourse.alu_op_type import AluOpType as Op
    AF = mybir.ActivationFunctionType

    nc = bacc.Bacc("TRN2", target_bir_lowering=False, debug=False)
    x_d = nc.dram_tensor("x", [P, L], mybir.dt.float32, kind="ExternalInput")
    aux_d = nc.dram_tensor("aux", [P, 8], mybir.dt.float32, kind="ExternalInput")
    shm_d = nc.dram_tensor("shm", [P, P], mybir.dt.float32, kind="ExternalInput")
    y_d = nc.dram_tensor("y", [P, L], mybir.dt.float32, kind="ExternalOutput")

    f32 = mybir.dt.float32
    with TileContext(nc) as tc:
        with tc.tile_pool(name="pool", bufs=1) as pool, \
             tc.tile_pool(name="psum", bufs=1, space="PSUM") as psum:
            tx = pool.tile([P, L], f32)        # x (resident; used at the end)
            tv = pool.tile([P, L], f32)        # v; post: u = r + v
            tD = pool.tile([P, L], f32)        # delta; post: |w|, exp scratch
            tse = pool.tile([P, L + 1], f32)   # r trajectory, col0 = carry
            ta = pool.tile([P, L], f32)        # coefficients a; post: gg, y
            taux = pool.tile([P, 8], f32)      # host-computed columns
            tshm = pool.tile([P, P], f32)      # carry shift matrix (PE)
            pinit = psum.tile([P, 1], f32)     # shifted carries via PE

            LNB = taux[:, 0:1]    # 1e-8 (Ln bias)
            PRB = taux[:, 1:2]    # 2*CUP*th (Prelu bias)
            EXS = taux[:, 2:3]    # dep (Exp scale)
            CLK = taux[:, 4:5]    # exp(dep*UPR) clamp
            TCL = taux[:, 5:6]    # v at end of previous chunk (host ln)
            SC2 = taux[:, 7:8]    # scratch

            nc.sync.dma_start(taux[:], aux_d[:])
            for j in range(NS):
                sl = slice(j * CW, (j + 1) * CW)
                nc.sync.dma_start(tx[:, sl], x_d[:, sl])
            nc.sync.dma_start(tshm[:], shm_d[:])

            # preload the natural_log table set (contains Square/Sign/Copy
            # too) before the first streamed Square lands
            nc.scalar.activation(SC2, taux[:, 6:7], AF.Ln, bias=LNB, scale=1.0)

            # scan-carry column starts at 0 (r_{-1} = 0)
            nc.vector.memset(tse[:, 0:1], 0.0)

            # streamed: v' = ln(x^2 + 1e-16) = 2*ln(|x|+1e-8) on ACT only
            # (Square then Ln; the factor 2 is absorbed exactly into the
            # downstream constants), delta on DVE, a0 = A_REL + DA*[delta<0]
            # on DVE per half as soon as its slices are in.
            for j in range(NS):
                sl = slice(j * CW, (j + 1) * CW)
                nc.scalar.activation(tv[:, sl], tx[:, sl], AF.Square, bias=0.0, scale=1.0)
                nc.scalar.activation(tv[:, sl], tv[:, sl], AF.Ln, bias=LNB, scale=1.0)
                lo = j * CW
                s_in = slice(lo if j else 1, (j + 1) * CW)
                s_sh = slice((lo - 1) if j else 0, (j + 1) * CW - 1)
                nc.vector.tensor_tensor(tD[:, s_in], tv[:, s_sh], tv[:, s_in],
                                        Op.subtract)
                if j == 0:
                    # cross-chunk delta col 0 from the host-computed column
                    nc.vector.tensor_tensor(tD[:, 0:1], TCL, tv[:, 0:1],
                                            Op.subtract)
                if j == NS // 2 - 1:
                    # a0 first half on DVE (idle while slices stream in)
                    nc.vector.tensor_scalar(ta[:, 0:H], tD[:, 0:H], 0.0, None,
                                            op0=Op.is_lt)
                    nc.vector.tensor_scalar(ta[:, 0:H], ta[:, 0:H], DA, A_REL,
                                            op0=Op.mult, op1=Op.add)
                if j == NS - 1:
                    # a0 second half on ACT: delta h2 is complete before
                    # scan h1 finishes, so this hides in the ACT idle window
                    nc.scalar.activation(ta[:, H:L], tD[:, H:L], AF.Sign,
                                         bias=0.0, scale=1.0)
                    nc.scalar.activation(ta[:, H:L], ta[:, H:L], AF.Copy,
                                         bias=MID, scale=-HDA)

            # dummy Exp chained after the last Ln: pulls the switch to the
            # exp_and_others table set (which also has Sign/Copy/Abs) into
            # the ACT idle window instead of the gain tail
            nc.scalar.activation(SC2, tv[:, L - 1:L], AF.Exp, bias=0.0, scale=0.0)

            for it in range(NITER - 1):
                init = tse[:, 0:1] if it == 0 else pinit[:]
                nc.vector.tensor_tensor_scan(
                    tse[:, 1:H + 1], tD[:, 0:H], ta[:, 0:H], init,
                    op0=Op.add, op1=Op.mult)
                # next modes for h1 overlap the h2 scan on the DVE
                nc.scalar.activation(ta[:, 0:H], tse[:, 1:H + 1], AF.Sign,
                                     bias=0.0, scale=1.0)
                nc.scalar.activation(ta[:, 0:H], ta[:, 0:H], AF.Copy,
                                     bias=MID, scale=-HDA)
                nc.vector.tensor_tensor_scan(
                    tse[:, H + 1:L + 1], tD[:, H:L], ta[:, H:L], tse[:, H:H + 1],
                    op0=Op.add, op1=Op.mult)
                nc.scalar.activation(ta[:, H:L], tse[:, H + 1:L + 1], AF.Sign,
                                     bias=0.0, scale=1.0)
                nc.scalar.activation(ta[:, H:L], ta[:, H:L], AF.Copy,
                                     bias=MID, scale=-HDA)
                # lagged chunk carries: shift partition p-1 -> p on the
                # idle Tensor engine (rows 0/63 get 0), result in PSUM
                nc.tensor.matmul(pinit[:], tshm[:], tse[:, L:L + 1])

            # carry for the final (s-form) iteration: s_init = shifted
            # r_end + shifted v_end; the latter is the host TCL column
            # (which also holds v_0 for each row's chunk 0, matching the
            # s_init = v_0 convention there since shifted r_end is 0)
            scol = taux[:, 3:4]
            nc.vector.tensor_tensor(scol, pinit[:], TCL, Op.add)

            # ------------- final iteration + gain tail, interleaved -------
            #   u = r + v;  w = u - th
            #   g = c1*w + c2*|w|  ==  Prelu(-2*CUP*w; alpha=-CDN/CUP)
            #   y = min(exp(dep*g), e^{dep*UPR}) * x
            # The last iteration's scan runs in quarters; each quarter's
            # gain chain (TT u -> ACT Prelu -> ACT Exp -> STT y -> DMA)
            # starts while later quarters are still scanning, so the y
            # output DMA streams during the final scans.
            # piece boundaries: big pieces early, small pieces last so the
            # final serial chain (scan->Prelu->Exp->y) is short.  The final
            # iteration scans s directly:  s = a*s - (a-1)*v  (op0=mult,
            # op1=subtract, data1 = (a-1)*v precomputed per piece), so no
            # u = r + v pass is needed and Prelu reads the trajectory.
            cuts = [0, 1750, 3500, 5250, 6126, 7000]
            pend = []   # y pieces whose Exp is in flight
            for i in range(len(cuts) - 1):
                lo, hi = cuts[i], cuts[i + 1]
                sl = slice(lo, hi)
                nc.vector.scalar_tensor_tensor(
                    tD[:, sl], ta[:, sl], 1.0, tv[:, sl], op0=Op.subtract, op1=Op.mult)
                init = scol if i == 0 else tse[:, lo:lo + 1]
                nc.vector.tensor_tensor_scan(
                    tse[:, lo + 1:hi + 1], ta[:, sl], tD[:, sl], init,
                    op0=Op.mult, op1=Op.subtract)
                nc.scalar.activation(tD[:, sl], tse[:, lo + 1:hi + 1], AF.Prelu,
                                     bias=PRB, scale=-CUP, alpha=ALF)
                nc.scalar.activation(tD[:, sl], tD[:, sl], AF.Exp, bias=0.0, scale=EXS)
                if hi - lo > 1000:
                    pend.append(slice(lo, (lo + hi) // 2))
                    pend.append(slice((lo + hi) // 2, hi))
                else:
                    pend.append(sl)
                if i >= 1:
                    se = pend.pop(0)
                    nc.vector.scalar_tensor_tensor(
                        ta[:, se], tD[:, se], CLK, tx[:, se], op0=Op.min, op1=Op.mult)
                    nc.sync.dma_start(y_d[:, se], ta[:, se])
            for se in pend:
                nc.vector.scalar_tensor_tensor(
                    ta[:, se], tD[:, se], CLK, tx[:, se], op0=Op.min, op1=Op.mult)
                nc.sync.dma_start(y_d[:, se], ta[:, se])

    nc.compile()
    return nc


_NC = None


def _get_nc():
    global _NC
    if _NC is None:
        _NC = build_nc()
    return _NC


def _shift_matrix():
    """W[k, p] = 1 iff p = k+1 within a row's chunk run (chunk 0 gets 0)."""
    w = np.zeros((P, P), np.float32)
    for p in range(P):
        if p % NCH != 0:
            w[p - 1, p] = 1.0
    return w


_SHM = _shift_matrix()


def make_in_maps(x, threshold, depth):
    th_nat = ((TMIN + threshold.astype(np.float32) * (TMAX - TMIN)) *
              np.float32(CNAT)).astype(np.float32)           # [16,1]
    dep = depth.astype(np.float32)
    aux_full = np.zeros((B, 8), np.float32)
    aux_full[:, 0] = 1e-16
    aux_full[:, 1] = np.float32(2.0 * CUP) * th_nat[:, 0]
    aux_full[:, 2] = dep[:, 0]
    aux_full[:, 4] = np.exp(dep[:, 0] * np.float32(UPR))
    in_maps = []
    for i in range(NCORES):
        xs = np.ascontiguousarray(x[ROWS * i:ROWS * (i + 1)]).reshape(P, L)
        auxs = np.repeat(aux_full[ROWS * i:ROWS * (i + 1)], NCH, axis=0)
        # host-computed v at end of previous chunk (chunk 0: own col 0,
        # so delta col0 = 0 there)
        vend = np.log(xs[:, L - 1] ** 2 + np.float32(1e-16)).astype(np.float32)
        tcl = np.empty(P, np.float32)
        tcl[1:] = vend[:-1]
        for r0 in (0, NCH):
            tcl[r0] = np.log(xs[r0, 0] ** 2 + np.float32(1e-16))
        auxs = np.ascontiguousarray(auxs, np.float32)
        auxs[:, 5] = tcl
        in_maps.append({"x": xs.astype(np.float32), "aux": auxs, "shm": _SHM})
    return in_maps


def kernel(x, threshold, depth):
    _install_ntff_hook()
    from concourse.bass_utils import run_bass_kernel_spmd
    nc = _get_nc()
    x = np.asarray(x, np.float32)
    in_maps = make_in_maps(x, np.asarray(threshold), np.asarray(depth))
    res = run_bass_kernel_spmd(nc, in_maps, core_ids=list(range(NCORES)))
    y = np.empty((B, N), np.float32)
    for i in range(NCORES):
        y[ROWS * i:ROWS * (i + 1)] = np.asarray(res.results[i]["y"]).reshape(ROWS, N)
    return y


# revision 22
# speedup vs baseline: 1.0526x; 1.0022x over previous
"""Trainium2 Bass kernel for the differentiable compressor.

Algorithm
---------
The time recurrence  s_t = a_t s_{t-1} + (1-a_t) v_t,
a_t = A_AT if v_t > s_{t-1} else A_REL  is a max-linear system; policy
iteration (guess modes, solve the linear recurrence exactly with the
hardware tensor_tensor_scan, repeat) converges geometrically.  4 total
iterations reach ~7.5e-4 output rel err (gate 2e-2).

Everything runs in natural-log units (v = ln(|x|+1e-8)).  Trajectory kept
relative to the input, r_t = s_t - v_t, so the scan is
    r_t = a_t * (r_{t-1} + delta_t),   delta_t = v_{t-1} - v_t,
with delta precomputed once.  Key identity: sign(r_t) = sign(r_{t-1} +
delta_t), so the next iteration's modes come straight from the previous
trajectory's sign: a_t = MID - HDA * sign(r_t_prev), computed entirely on
the Scalar engine (Sign + Copy-affine) and hidden under the scans.

Gain stage: the knee-smoothed down/up gains collapse (to ~1e-4 rel) to
    g = c1*w + c2*|w|,  c1 = CDN-CUP, c2 = CDN+CUP,  w = r + v - th,
clamped at UP_RANGE on the up side (applied after exp, where it is a min
against e^{dep*UPR}).  Per quarter: TT (u = r+v), ACT Abs (bias=-th),
STT (gg = (c2/c1)|w| + u), ACT Exp (scale=c1*dep, bias=-c1*dep*th), and
STT ((eg min clamp) * x).  All activations used (Ln, Exp, Abs, Sign,
Copy) live in one table set -> one ACT_TABLE_LOAD total.

Layout per core: 2 batch rows x 441000 samples -> [126 partitions x 7000],
63 time-chunks per row.  Chunk-boundary carries live in an extra leading
column of the trajectory tile; between iterations the carry shift
(partition p-1 -> p) is done by the idle Tensor engine with a host-
supplied shift matrix into PSUM, off the DVE critical path.  The final
iteration also uses lagged carries (exact-carry fixup unneeded).

Sharding: pure data parallel, batch 16 -> 2 rows on each of 8 cores.
"""
import sys
import types
import numpy as np

# ---------------- constants (natural-log units) ----------------
SR = 44100.0
A_AT = float(np.exp(-1.0 / (10.0 * SR / 1000.0)))     # attack coeff
A_REL = float(np.exp(-1.0 / (100.0 * SR / 1000.0)))   # release coeff
DA = A_AT - A_REL
MID = (A_AT + A_REL) / 2.0
HDA = (A_AT - A_REL) / 2.0
CNAT = float(np.log(10.0) / 20.0)                     # dB -> nat
CDN = -(1.0 - 1.0 / 66.7) * 0.5                       # down-ratio gain slope
CUP = (1.0 - 0.1) * 0.5                               # up-ratio gain slope
C1 = CDN - CUP
C2 = CDN + CUP
C21 = C2 / C1
ALF = (C1 + C2) / (C1 - C2)   # Prelu negative-side slope = -CDN/CUP
UPR = 36.0 * CNAT                                     # up-range clamp
TMIN, TMAX = -40.0, 0.0

B, N = 16, 441000
NCORES = 8
ROWS = 2           # batch rows per core
NCH = 63           # chunks per row
P = ROWS * NCH     # 126 partitions
L = N // NCH       # 7000 chunk length
H = L // 2         # half-width for engine overlap
Q = L // 4         # quarter-width for the gain tail

NITER = 4          # policy iterations (incl. the first delta-sign one)
NS = 10            # x streaming slices (700 cols each, even for 2x DVE mode)
CW = L // NS


def _install_ntff_hook():
    """Inject the missing antenv.axon_hooks so trace=True profiling works."""
    try:
        import antenv
        if "antenv.axon_hooks" not in sys.modules:
            m = types.ModuleType("antenv.axon_hooks")
            m._hook = None
            def _set(h, _m=m): _m._hook = h
            def _get(_m=m): return _m._hook
            m.set_axon_ntff_profile_hook = _set
            m.get_axon_ntff_profile_hook = _get
            sys.modules["antenv.axon_hooks"] = m
            antenv.axon_hooks = m
            from trn_agent_boot.trn_boot import _ntff_profile_via_ctypes
            _set(_ntff_profile_via_ctypes("/opt/axon/libaxon_pjrt.so"))
    except Exception:
        pass


def build_nc():
    import concourse.bacc as bacc
    import concourse.mybir as mybir
    from concourse.tile import TileContext
    from concourse.alu_op_type import AluOpType as Op
    AF = mybir.ActivationFunctionType

    nc = bacc.Bacc("TRN2", target_bir_lowering=False, debug=False)
    x_d = nc.dram_tensor("x", [P, L], mybir.dt.float32, kind="ExternalInput")
    aux_d = nc.dram_tensor("aux", [P, 8], mybir.dt.float32, kind="ExternalInput")
    shm_d = nc.dram_tensor("shm", [P, P], mybir.dt.float32, kind="ExternalInput")
    y_d = nc.dram_tensor("y", [P, L], mybir.dt.float32, kind="ExternalOutput")

    f32 = mybir.dt.float32
    with TileContext(nc) as tc:
        with tc.tile_pool(name="pool", bufs=1) as pool, \
             tc.tile_pool(name="psum", bufs=1, space="PSUM") as psum:
            tx = pool.tile([P, L], f32)        # x (resident; used at the end)
            tv = pool.tile([P, L], f32)        # v; post: u = r + v
            tD = pool.tile([P, L], f32)        # delta; post: |w|, exp scratch
            tse = pool.tile([P, L + 1], f32)   # r trajectory, col0 = carry
            ta = pool.tile([P, L], f32)        # coefficients a; post: gg, y
            taux = pool.tile([P, 8], f32)      # host-computed columns
            tshm = pool.tile([P, P], f32)      # carry shift matrix (PE)
            pinit = psum.tile([P, 1], f32)     # shifted carries via PE

            LNB = taux[:, 0:1]    # 1e-8 (Ln bias)
            PRB = taux[:, 1:2]    # 2*CUP*th (Prelu bias)
            EXS = taux[:, 2:3]    # dep (Exp scale)
            CLK = taux[:, 4:5]    # exp(dep*UPR) clamp
            TCL = taux[:, 5:6]    # v at end of previous chunk (host ln)
            SC2 = taux[:, 7:8]    # scratch

            nc.sync.dma_start(taux[:], aux_d[:])
            for j in range(NS):
                sl = slice(j * CW, (j + 1) * CW)
                nc.sync.dma_start(tx[:, sl], x_d[:, sl])
            nc.sync.dma_start(tshm[:], shm_d[:])

            # preload the natural_log table set (contains Square/Sign/Copy
            # too) before the first streamed Square lands
            nc.scalar.activation(SC2, taux[:, 6:7], AF.Ln, bias=LNB, scale=1.0)

            # scan-carry column starts at 0 (r_{-1} = 0)
            nc.vector.memset(tse[:, 0:1], 0.0)

            # streamed: v' = ln(x^2 + 1e-16) = 2*ln(|x|+1e-8) on ACT only
            # (Square then Ln; the factor 2 is absorbed exactly into the
            # downstream constants), delta on DVE, a0 = A_REL + DA*[delta<0]
            # on DVE per half as soon as its slices are in.
            for j in range(NS):
                sl = slice(j * CW, (j + 1) * CW)
                if j < 4:
                    # early slices: square on the idle DVE so the ACT
                    # stream chain (which gates a0-h2 late in the stream)
                    # only pays one op per slice
                    nc.vector.tensor_tensor(tv[:, sl], tx[:, sl], tx[:, sl],
                                            Op.mult)
                else:
                    nc.scalar.activation(tv[:, sl], tx[:, sl], AF.Square,
                                         bias=0.0, scale=1.0)
                nc.scalar.activation(tv[:, sl], tv[:, sl], AF.Ln, bias=LNB, scale=1.0)
                lo = j * CW
                s_in = slice(lo if j else 1, (j + 1) * CW)
                s_sh = slice((lo - 1) if j else 0, (j + 1) * CW - 1)
                nc.vector.tensor_tensor(tD[:, s_in], tv[:, s_sh], tv[:, s_in],
                                        Op.subtract)
                if j == 0:
                    # cross-chunk delta col 0 from the host-computed column
                    nc.vector.tensor_tensor(tD[:, 0:1], TCL, tv[:, 0:1],
                                            Op.subtract)
                if j == NS // 2 - 1:
                    # a0 first half on DVE (idle while slices stream in)
                    nc.vector.tensor_scalar(ta[:, 0:H], tD[:, 0:H], 0.0, None,
                                            op0=Op.is_lt)
                    nc.vector.tensor_scalar(ta[:, 0:H], ta[:, 0:H], DA, A_REL,
                                            op0=Op.mult, op1=Op.add)
                if j == NS - 1:
                    # a0 second half on ACT: delta h2 is complete before
                    # scan h1 finishes, so this hides in the ACT idle window
                    nc.scalar.activation(ta[:, H:L], tD[:, H:L], AF.Sign,
                                         bias=0.0, scale=1.0)
                    nc.scalar.activation(ta[:, H:L], ta[:, H:L], AF.Copy,
                                         bias=MID, scale=-HDA)

            # dummy Exp chained after the last Ln: pulls the switch to the
            # exp_and_others table set (which also has Sign/Copy/Abs) into
            # the ACT idle window instead of the gain tail
            nc.scalar.activation(SC2, tv[:, L - 1:L], AF.Exp, bias=0.0, scale=0.0)

            for it in range(NITER - 1):
                init = tse[:, 0:1] if it == 0 else pinit[:]
                nc.vector.tensor_tensor_scan(
                    tse[:, 1:H + 1], tD[:, 0:H], ta[:, 0:H], init,
                    op0=Op.add, op1=Op.mult)
                # next modes for h1 overlap the h2 scan on the DVE
                nc.scalar.activation(ta[:, 0:H], tse[:, 1:H + 1], AF.Sign,
                                     bias=0.0, scale=1.0)
                nc.scalar.activation(ta[:, 0:H], ta[:, 0:H], AF.Copy,
                                     bias=MID, scale=-HDA)
                nc.vector.tensor_tensor_scan(
                    tse[:, H + 1:L + 1], tD[:, H:L], ta[:, H:L], tse[:, H:H + 1],
                    op0=Op.add, op1=Op.mult)
                nc.scalar.activation(ta[:, H:L], tse[:, H + 1:L + 1], AF.Sign,
                                     bias=0.0, scale=1.0)
                nc.scalar.activation(ta[:, H:L], ta[:, H:L], AF.Copy,
                                     bias=MID, scale=-HDA)
                # lagged chunk carries: shift partition p-1 -> p on the
                # idle Tensor engine (rows 0/63 get 0), result in PSUM
                nc.tensor.matmul(pinit[:], tshm[:], tse[:, L:L + 1])

            # carry for the final (s-form) iteration: s_init = shifted
            # r_end + shifted v_end; the latter is the host TCL column
            # (which also holds v_0 for each row's chunk 0, matching the
            # s_init = v_0 convention there since shifted r_end is 0)
            scol = taux[:, 3:4]
            nc.vector.tensor_tensor(scol, pinit[:], TCL, Op.add)

            # ------------- final iteration + gain tail, interleaved -------
            #   u = r + v;  w = u - th
            #   g = c1*w + c2*|w|  ==  Prelu(-2*CUP*w; alpha=-CDN/CUP)
            #   y = min(exp(dep*g), e^{dep*UPR}) * x
            # The last iteration's scan runs in quarters; each quarter's
            # gain chain (TT u -> ACT Prelu -> ACT Exp -> STT y -> DMA)
            # starts while later quarters are still scanning, so the y
            # output DMA streams during the final scans.
            # piece boundaries: big pieces early, small pieces last so the
            # final serial chain (scan->Prelu->Exp->y) is short.  The final
            # iteration scans s directly:  s = a*s - (a-1)*v  (op0=mult,
            # op1=subtract, data1 = (a-1)*v precomputed per piece), so no
            # u = r + v pass is needed and Prelu reads the trajectory.
            cuts = [0, 1750, 3500, 5250, 6126, 7000]
            pend = []   # y pieces whose Exp is in flight
            for i in range(len(cuts) - 1):
                lo, hi = cuts[i], cuts[i + 1]
                sl = slice(lo, hi)
                nc.vector.scalar_tensor_tensor(
                    tD[:, sl], ta[:, sl], 1.0, tv[:, sl], op0=Op.subtract, op1=Op.mult)
                init = scol if i == 0 else tse[:, lo:lo + 1]
                nc.vector.tensor_tensor_scan(
                    tse[:, lo + 1:hi + 1], ta[:, sl], tD[:, sl], init,
                    op0=Op.mult, op1=Op.subtract)
                nc.scalar.activation(tD[:, sl], tse[:, lo + 1:hi + 1], AF.Prelu,
                                     bias=PRB, scale=-CUP, alpha=ALF)
                nc.scalar.activation(tD[:, sl], tD[:, sl], AF.Exp, bias=0.0, scale=EXS)
                if hi - lo > 1000:
                    pend.append(slice(lo, (lo + hi) // 2))
                    pend.append(slice((lo + hi) // 2, hi))
                else:
                    pend.append(sl)
                if i >= 1:
                    se = pend.pop(0)
                    nc.vector.scalar_tensor_tensor(
                        ta[:, se], tD[:, se], CLK, tx[:, se], op0=Op.min, op1=Op.mult)
                    nc.sync.dma_start(y_d[:, se], ta[:, se])
            for se in pend:
                nc.vector.scalar_tensor_tensor(
                    ta[:, se], tD[:, se], CLK, tx[:, se], op0=Op.min, op1=Op.mult)
                nc.sync.dma_start(y_d[:, se], ta[:, se])

    nc.compile()
    return nc


_NC = None


def _get_nc():
    global _NC
    if _NC is None:
        _NC = build_nc()
    return _NC


def _shift_matrix():
    """W[k, p] = 1 iff p = k+1 within a row's chunk run (chunk 0 gets 0)."""
    w = np.zeros((P, P), np.float32)
    for p in range(P):
        if p % NCH != 0:
            w[p - 1, p] = 1.0
    return w


_SHM = _shift_matrix()


def make_in_maps(x, threshold, depth):
    th_nat = ((TMIN + threshold.astype(np.float32) * (TMAX - TMIN)) *
              np.float32(CNAT)).astype(np.float32)           # [16,1]
    dep = depth.astype(np.float32)
    aux_full = np.zeros((B, 8), np.float32)
    aux_full[:, 0] = 1e-16
    aux_full[:, 1] = np.float32(2.0 * CUP) * th_nat[:, 0]
    aux_full[:, 2] = dep[:, 0]
    aux_full[:, 4] = np.exp(dep[:, 0] * np.float32(UPR))
    in_maps = []
    for i in range(NCORES):
        xs = np.ascontiguousarray(x[ROWS * i:ROWS * (i + 1)]).reshape(P, L)
        auxs = np.repeat(aux_full[ROWS * i:ROWS * (i + 1)], NCH, axis=0)
        # host-computed v at end of previous chunk (chunk 0: own col 0,
        # so delta col0 = 0 there)
        vend = np.log(xs[:, L - 1] ** 2 + np.float32(1e-16)).astype(np.float32)
        tcl = np.empty(P, np.float32)
        tcl[1:] = vend[:-1]
        for r0 in (0, NCH):
            tcl[r0] = np.log(xs[r0, 0] ** 2 + np.float32(1e-16))
        auxs = np.ascontiguousarray(auxs, np.float32)
        auxs[:, 5] = tcl
        in_maps.append({"x": xs.astype(np.float32), "aux": auxs, "shm": _SHM})
    return in_maps


def kernel(x, threshold, depth):
    _install_ntff_hook()
    from concourse.bass_utils import run_bass_kernel_spmd
    nc = _get_nc()
    x = np.asarray(x, np.float32)
    in_maps = make_in_maps(x, np.asarray(threshold), np.asarray(depth))
    res = run_bass_kernel_spmd(nc, in_maps, core_ids=list(range(NCORES)))
    y = np.empty((B, N), np.float32)
    for i in range(NCORES):
        y[ROWS * i:ROWS * (i + 1)] = np.asarray(res.results[i]["y"]).reshape(ROWS, N)
    return y
